# revision 12
# baseline (speedup 1.0000x reference)
"""Trainium2 Bass kernel for nn_CausalAttention (GNN message passing).

Math (reference):
    pairs[e] = [img[:, src[e]] ; text[:, tgt[e]]]          # B == H == 128
    a[e]     = sigmoid(w2 . relu(W1 @ pairs[e] + b1) + b2) # per-edge gate
    att_img[b, i] = sum_{e: src[e]=i} a[e] * text[b, tgt[e]]
    att_txt[b, t] = sum_{e: tgt[e]=t} a[e] * img[b, src[e]]

Architecture: output-column sharding, on-chip one-hot matmul gathers/
scatters, single fp16 precision (tolerance 2e-2, this lands ~4e-4).
One-hot matrices ship from host as fp8e4 (exact for 0/1, halves DMA);
the PE accepts mixed fp16 x fp8 operands (probed bit-exact on HW).

Core c owns att_img[:, Wc], att_txt[:, Wc], Wc = [128c, 128c+128).
Per pipe (img shown; txt symmetric, roles swapped):
  - edges with src in Wc, bucketed by w = tgt >> 7 (8 buckets x 5
    blocks x 128 slots; dummy slots have key -1 -> all-zero one-hots).
  - phase A (PE): h = relu(UwinT.T @ ohKT + V8[w].T @ ohLT + b1);
    per-block N=1 matmuls transpose w2.h into [e%128, blk] layout;
    sigmoid -> a [128, 40].
  - phase B: ohKa_bucket = ohK_bucket * broadcast(a) (one DVE op per
    bucket); PE scatters M_w[lo, loc] += ohlo.T @ ohKa (PSUM accum).
  - tail: att[:, loc] = sum_w txtT8[w].T @ M_w (8 fp16 matmuls).
Scheduling: dummy warm-up matmuls ramp the PE p-state and dummy
relu/sigmoid preload both activation tables during the DMA window;
DMAs are issued on both HWDGE queues (sync + scalar) interleaved in
need order imgA -> imgB -> txtA -> txtB -> tails.
Host concatenates the 8 column slices of each output.
"""

import sys

for _p in ("/opt/trn_rl_repo", "/root/.axon_site/_ro/trn_rl_repo"):
    if _p not in sys.path:
        sys.path.insert(0, _p)

import numpy as np

import concourse.tile as tile
from concourse import bacc, mybir

P = 128
DIM = 1024
NCORES = 8
NW = 8            # hi buckets
BPW = 5           # blocks per bucket (capacity 640 vs mean 512)
NBLK = NW * BPW   # 40
EC = NBLK * P     # 5120 edge slots per pipeline
BW = BPW * P      # 640 edges per bucket
HALF = EC // 2    # 2560 one-hot cols per DMA chunk
NWARM = 14        # PE p-state warm-up matmuls

F32 = mybir.dt.float32
F16 = mybir.dt.float16
F8 = mybir.dt.float8e4
OH_NP = mybir.dt.np(F8)

MULT = mybir.AluOpType.mult
RELU = mybir.ActivationFunctionType.Relu
SIGM = mybir.ActivationFunctionType.Sigmoid

# cpk1 column layout (fp16): small, UV-build critical
C_W2 = 0
C_W1I = 1
C_W1X = C_W1I + P
C_IWIN = C_W1X + P
C_TWIN = C_IWIN + P
C1_TOT = C_TWIN + P         # 513
# cpk2: txt first (V8 is needed before U8)
C_TXT = 0
C_IMG = DIM
C2_TOT = 2 * DIM


def _build_program():
    nc = bacc.Bacc(None, target_bir_lowering=False, debug=False)

    cpk1 = nc.dram_tensor("cpk1", [P, C1_TOT], F16, kind="ExternalInput")
    cpk2 = nc.dram_tensor("cpk2", [P, C2_TOT], F16, kind="ExternalInput")
    tpk = nc.dram_tensor("tpk", [P, 2 * DIM], F16, kind="ExternalInput")
    mpk = nc.dram_tensor("mpk", [P, 2], F32, kind="ExternalInput")
    ohd = {}
    for s in ("i", "t"):
        for k in ("okt", "olt"):
            for h in range(2):
                nm = f"{s}_{k}{h}"
                ohd[nm] = nc.dram_tensor(nm, [P, HALF], F8, kind="ExternalInput")
        for k in ("olo", "okk"):
            nm = f"{s}_{k}"
            ohd[nm] = nc.dram_tensor(nm, [P, EC], F8, kind="ExternalInput")
    out_img = nc.dram_tensor("out_img", [P, P], F32, kind="ExternalOutput")
    out_txt = nc.dram_tensor("out_txt", [P, P], F32, kind="ExternalOutput")

    with tile.TileContext(nc) as tc:
        with (
            tc.tile_pool(name="const", bufs=1) as cp,
            tc.tile_pool(name="work", bufs=3) as wp,
            tc.tile_pool(name="ka", bufs=3) as kp,
            tc.tile_pool(name="psH", bufs=2, space="PSUM") as psH,
            tc.tile_pool(name="psM", bufs=1, space="PSUM") as psM,
            tc.tile_pool(name="psS", bufs=1, space="PSUM") as psS,
        ):
            cpk1_s = cp.tile([P, C1_TOT], F16)
            cpk2_s = cp.tile([P, C2_TOT], F16)
            tpk_s = cp.tile([P, 2 * DIM], F16)
            mpk_s = cp.tile([P, 2], F32)
            warm_s = cp.tile([P, P], F16)
            U8 = cp.tile([P, NW, P], F16)
            V8 = cp.tile([P, NW, P], F16)
            winT2 = cp.tile([P, 2 * P], F16)
            oh_s = {}
            for s in ("i", "t"):
                for k in ("okt", "olt"):
                    for h in range(2):
                        nm = f"{s}_{k}{h}"
                        oh_s[nm] = cp.tile([P, HALF], F8, tag=nm, name=nm)
                for k in ("olo", "okk"):
                    nm = f"{s}_{k}"
                    oh_s[nm] = cp.tile([P, EC], F8, tag=nm, name=nm)

            # ---- DMA issue, both HWDGE queues, interleaved in need
            # order: UV -> A(img) -> B(img) -> A(txt) -> B(txt) -> tails
            nc.sync.dma_start(cpk1_s[:], cpk1[:])
            nc.scalar.dma_start(mpk_s[:], mpk[:])
            nc.sync.dma_start(cpk2_s[:], cpk2[:])
            for nm in ("i_okt0", "i_okt1", "i_olo", "t_okt0", "t_okt1",
                       "t_olo"):
                nc.sync.dma_start(oh_s[nm][:], ohd[nm][:])
            for nm in ("i_olt0", "i_olt1", "i_okk", "t_olt0", "t_olt1",
                       "t_okk"):
                nc.scalar.dma_start(oh_s[nm][:], ohd[nm][:])
            nc.scalar.dma_start(tpk_s[:], tpk[:])

            w2_s = cpk1_s[:, C_W2 : C_W2 + 1]
            w1i_s = cpk1_s[:, C_W1I : C_W1I + P]
            w1x_s = cpk1_s[:, C_W1X : C_W1X + P]
            iwin_s = cpk1_s[:, C_IWIN : C_IWIN + P]
            twin_s = cpk1_s[:, C_TWIN : C_TWIN + P]
            txt_s = cpk2_s[:, C_TXT : C_TXT + DIM]
            img_s = cpk2_s[:, C_IMG : C_IMG + DIM]
            UwinT = winT2[:, :P]
            VwinT = winT2[:, P:]
            b1_s = mpk_s[:, 0:1]
            b2_s = mpk_s[:, 1:2]

            # ---- PE p-state warm-up on junk data; also preload both
            # activation tables (relu+sigmoid) off the critical path ----
            nc.gpsimd.memset(warm_s[:], 0.0)
            warm_ps = psH.tile([P, BW], F32, tag="h_ps", name="warm_ps")
            for i in range(NWARM):
                nc.tensor.matmul(warm_ps[:, :P], warm_s[:], warm_s[:],
                                 start=True, stop=True, skip_group_check=True)
            dum = wp.tile([P, 1], F32, tag="dum")
            nc.scalar.activation(dum[:], warm_s[:, :1], RELU, bias=0.0)
            dum2 = wp.tile([P, 1], F32, tag="dum")
            nc.scalar.activation(dum2[:], warm_s[:, :1], SIGM, bias=0.0)

            # ---- U/V tables via psM banks (free until phase B):
            # V8[:, w, :] = (txt block w).T @ W1x.T, batched f32->f16 casts
            uv0 = psM.tile([P, 4 * P], F32, tag="m0", name="uv0")
            nc.tensor.matmul(uv0[:, :P], iwin_s, w1i_s, start=True, stop=True,
                             skip_group_check=True)
            nc.tensor.matmul(uv0[:, P : 2 * P], twin_s, w1x_s, start=True,
                             stop=True, skip_group_check=True)
            nc.vector.tensor_copy(winT2[:], uv0[:, : 2 * P])

            for tab, src_, w1_ in ((V8, txt_s, w1x_s), (U8, img_s, w1i_s)):
                for hf in range(2):
                    ps = psM.tile([P, 4 * P], F32, tag="m1", name=f"uv{hf}")
                    for w4 in range(4):
                        w = hf * 4 + w4
                        nc.tensor.matmul(
                            ps[:, w4 * P : (w4 + 1) * P],
                            src_[:, w * P : (w + 1) * P], w1_,
                            start=True, stop=True, skip_group_check=True,
                        )
                    nc.vector.tensor_copy(
                        tab[:, hf * 4 : (hf + 1) * 4, :], ps[:]
                    )

            sides = (("i", UwinT, V8, 0, out_img),
                     ("t", VwinT, U8, DIM, out_txt))
            a_sb = {}
            a_ps2 = psS.tile([P, 2 * NBLK], F32, tag="a_ps2", name="a_ps2")
            acc2 = psS.tile([P, 2 * P], F32, tag="acc2", name="acc2")

            # ---- phase A both pipes: per-edge gate a ----
            for si, (side, winT, arb8, _t8o, _od) in enumerate(sides):
                a_ps = a_ps2[:, si * NBLK : (si + 1) * NBLK]
                for w in range(NW):
                    e0 = w * BW
                    h_ = 0 if w < 4 else 1
                    c0 = e0 - h_ * HALF
                    ohKT = oh_s[f"{side}_okt{h_}"][:, c0 : c0 + BW]
                    ohLT = oh_s[f"{side}_olt{h_}"][:, c0 : c0 + BW]
                    h_ps = psH.tile([P, BW], F32, tag="h_ps")
                    for mi, (st, oh_) in enumerate(
                        ((winT, ohKT), (arb8[:, w, :], ohLT))
                    ):
                        for o, n in ((0, 4 * P), (4 * P, P)):
                            nc.tensor.matmul(
                                h_ps[:, o : o + n], st, oh_[:, o : o + n],
                                start=(mi == 0), stop=(mi == 1),
                            )
                    h_s = wp.tile([P, BW], F16, tag="h_s")
                    nc.scalar.activation(h_s[:], h_ps[:], RELU, bias=b1_s)
                    for j in range(BPW):
                        b = w * BPW + j
                        nc.tensor.matmul(
                            a_ps[:, b : b + 1], h_s[:, j * P : (j + 1) * P],
                            w2_s, start=True, stop=True,
                        )
                a_s = wp.tile([P, NBLK], F32, tag=f"a_s_{side}",
                              name=f"a_s_{side}")
                nc.scalar.activation(a_s[:], a_ps[:], SIGM, bias=b2_s)
                a_sb[side] = a_s

            # ---- phase B + tail per pipe (m_ps banks reused) ----
            for si, (side, _w, _a, t8off, out_d) in enumerate(sides):
                m_ps0 = psM.tile([P, 4 * P], F32, tag="m0", name=f"m0{side}")
                m_ps1 = psM.tile([P, 4 * P], F32, tag="m1", name=f"m1{side}")
                m_ps = [m_ps0, m_ps1]
                olo, okk = oh_s[f"{side}_olo"], oh_s[f"{side}_okk"]
                a_s = a_sb[side]
                for w in range(NW):
                    e0 = w * BW
                    ohKa = kp.tile([P, BW], F16, tag="ohKa")
                    nc.vector.tensor_tensor(
                        out=ohKa[:].rearrange("p (b l) -> p b l", b=BPW),
                        in0=okk[:, e0 : e0 + BW].rearrange(
                            "p (b l) -> p b l", b=BPW),
                        in1=a_s[:, w * BPW : (w + 1) * BPW].broadcast_to(
                            (P, BPW, P)),
                        op=MULT,
                    )
                    for j in range(BPW):
                        b = w * BPW + j
                        nc.tensor.matmul(
                            m_ps[w // 4][:, (w % 4) * P : (w % 4 + 1) * P],
                            olo[:, b * P : (b + 1) * P],
                            ohKa[:, j * P : (j + 1) * P],
                            start=(j == 0), stop=(j == BPW - 1),
                            skip_group_check=True,
                        )

                # tail: att[:, loc] = sum_w arbT8[w].T @ M_w
                acc = acc2[:, si * P : (si + 1) * P]
                for g in range(2):
                    m_s4 = wp.tile([P, 4 * P], F16, tag="m_s4")
                    nc.scalar.copy(m_s4[:], m_ps[g][:])
                    for w4 in range(4):
                        w = g * 4 + w4
                        nc.tensor.matmul(
                            acc,
                            tpk_s[:, t8off + w * P : t8off + (w + 1) * P],
                            m_s4[:, w4 * P : (w4 + 1) * P],
                            start=(w == 0), stop=(w == NW - 1),
                            skip_group_check=True,
                        )
                out_sb = wp.tile([P, P], F32, tag="out_sb")
                nc.vector.tensor_copy(out_sb[:], acc)
                nc.sync.dma_start(out_d[:], out_sb[:])

    nc.compile()
    return nc


_PROGRAM = None


def _get_program():
    global _PROGRAM
    if _PROGRAM is None:
        _PROGRAM = _build_program()
    return _PROGRAM


def _pipe_arrays(key, arb, base):
    """key: window-owning endpoint (src for img pipe); arb: other endpoint.
    Returns ohkt, ohlt [P, EC] (gather one-hots, [idx, e]) and
    ohlo, ohk [P, EC] (scatter one-hots, [e%128, blk*128+idx])."""
    kloc = key - base                 # 0..127
    w = arb >> 7                      # bucket
    lo = arb & 127
    slots = np.full(EC, -1, np.int64)  # slot -> edge index or -1
    fill = np.zeros(NW, np.int64)
    order = np.argsort(w, kind="stable")
    for ei in order:
        wb = w[ei]
        assert fill[wb] < BW, f"bucket overflow: {fill[wb]}"
        slots[wb * BW + fill[wb]] = ei
        fill[wb] += 1
    klocs = np.full(EC, -1, np.int64)
    los = np.full(EC, -1, np.int64)
    used = slots >= 0
    klocs[used] = kloc[slots[used]]
    los[used] = lo[slots[used]]
    rng = np.arange(P)
    ohkt = np.ascontiguousarray((klocs[None, :] == rng[:, None]).astype(OH_NP))
    ohlt = np.ascontiguousarray((los[None, :] == rng[:, None]).astype(OH_NP))
    # block-diagonal [e, idx] layouts for the scatter matmuls
    lob = los.reshape(NBLK, P).T      # [e%128, blk]
    klb = klocs.reshape(NBLK, P).T
    ohlo = np.zeros((P, NBLK, P), OH_NP)
    ohk = np.zeros((P, NBLK, P), OH_NP)
    ohlo[lob[:, :, None] == rng[None, None, :]] = OH_NP(1.0)
    ohk[klb[:, :, None] == rng[None, None, :]] = OH_NP(1.0)
    return (ohkt, ohlt,
            np.ascontiguousarray(ohlo.reshape(P, EC)),
            np.ascontiguousarray(ohk.reshape(P, EC)))


def _t8(x16):
    """[b, col] fp16 -> [lo, w*128 + b] with col = 128w + lo."""
    return np.ascontiguousarray(
        x16.T.reshape(NW, P, P).transpose(1, 0, 2).reshape(P, DIM)
    )


def _make_in_maps(img_features, text_features, src, tgt, W1, b1, w2, b2):
    img16 = img_features.astype(np.float16)
    txt16 = text_features.astype(np.float16)
    w1i16 = np.ascontiguousarray(W1[:, :P].T.astype(np.float16))
    w1x16 = np.ascontiguousarray(W1[:, P:].T.astype(np.float16))
    w2c16 = np.ascontiguousarray(w2.astype(np.float16).reshape(P, 1))
    b1c = np.ascontiguousarray(b1.astype(np.float32).reshape(P, 1))
    b2c = np.full((P, 1), np.float32(b2), dtype=np.float32)
    tpk = np.ascontiguousarray(
        np.concatenate([_t8(txt16), _t8(img16)], axis=1))
    src = np.asarray(src).astype(np.int64)
    tgt = np.asarray(tgt).astype(np.int64)

    in_maps = []
    for c in range(NCORES):
        base = c * P
        cpk1 = np.concatenate(
            [w2c16, w1i16, w1x16,
             img16[:, base : base + P], txt16[:, base : base + P]], axis=1)
        cpk2 = np.concatenate([txt16, img16], axis=1)
        m = {"cpk1": np.ascontiguousarray(cpk1),
             "cpk2": np.ascontiguousarray(cpk2), "tpk": tpk,
             "mpk": np.ascontiguousarray(np.concatenate([b1c, b2c], axis=1))}
        for s, key, arb in (("i", src, tgt), ("t", tgt, src)):
            sel = (key >= base) & (key < base + P)
            ohkt, ohlt, ohlo, ohk = _pipe_arrays(key[sel], arb[sel], base)
            m[f"{s}_okt0"] = np.ascontiguousarray(ohkt[:, :HALF])
            m[f"{s}_okt1"] = np.ascontiguousarray(ohkt[:, HALF:])
            m[f"{s}_olt0"] = np.ascontiguousarray(ohlt[:, :HALF])
            m[f"{s}_olt1"] = np.ascontiguousarray(ohlt[:, HALF:])
            m[f"{s}_olo"] = ohlo
            m[f"{s}_okk"] = ohk
        in_maps.append(m)
    return in_maps


def _run(inputs, trace=False):
    from concourse.bass_utils import run_bass_kernel_spmd

    nc = _get_program()
    in_maps = _make_in_maps(**inputs)
    res = run_bass_kernel_spmd(
        nc, in_maps, core_ids=list(range(NCORES)), trace=trace
    )
    att_img = np.concatenate([r["out_img"] for r in res.results], axis=1)
    att_txt = np.concatenate([r["out_txt"] for r in res.results], axis=1)
    return (np.ascontiguousarray(att_img), np.ascontiguousarray(att_txt)), res


def kernel(**inputs):
    out, _ = _run(inputs, trace=False)
    return out
